# revision 9
# baseline (speedup 1.0000x reference)
"""Trainium2 Bass kernel for nn_CorrAttentionBias.

Computes out = where(row/col masked, NEG, attn + neigh_band_bias + sink_bias)
for attn_scores [2, 16, 2048, 2048] f32, sharded over (batch, head) across
8 NeuronCores (4 heads of one batch per core).

Mask-aware packing: masked rows (~50%) have constant-NEG output and never
need their attn values, so the host packs only the unmasked rows (gathered,
head-transposed to [NU, H_PER, L] so each packed row is 32 KiB contiguous)
and the device:
  - reads the packed rows only          (~35 MB instead of 67 MB),
  - computes bias + col-mask on them    (halves the vector-engine work),
  - stores them contiguously to out[0:NU_max],
  - streams a constant NEG block to out[NU_max:] for the masked rows
    (pure DMA from SBUF, no compute dependencies).
The host unshard scatters rows back through the permutation; every output
byte is produced on device.

All DMA transfers use exactly 128 partitions: partial-partition descriptors
execute on a single DMA engine (~27 GB/s) instead of spreading across all
16, so the last block is anchored at nu_max-128 and overlapping rows are
computed/stored twice with identical bytes (same for the NEG region).

The program is compiled inside kernel() after the mask is known; it depends
on the mask only through block offsets and band column windows. SPMD across
cores requires one shape for both batches: block structure uses NU_max and
the smaller batch's packed buffer is padded (host ignores junk rows).

Per packed row-block of 128 rows (orig rows i = pi[k0 + p], sorted):
  bias[p, j] = round(BETA * round(cs[i]*cs[j]))            (ACT, 2 ops)
  bias[p, i-1] += sub[i]; bias[p, i+1] += sup[i]           (iota==t compare,
                               window-limited since pi is sorted)
  out[p, j] = min(attn[p, j] + bias[p, j], maskval[j])     (exact NEG via min)
All rounding matches the jax reference bitwise (NEG = -1e5 >> |attn+bias|).
"""

import sys

sys.path.insert(0, "/opt/trn_rl_repo")

from contextlib import ExitStack

import numpy as np

import concourse.bass as bass
import concourse.tile as tile
from concourse import bacc, mybir
from concourse.bass_utils import run_bass_kernel_spmd

ALPHA = np.float32(0.5)
BETA = np.float32(0.1)
NEG = np.float32(-100000.0)
BIG = np.float32(3.0e38)

B, H, L = 2, 16, 2048
N_CORES = 8
H_PER = (B * H) // N_CORES  # 4 heads per core
P = 128  # partitions per row-block
K = 5  # per-row values: cs, t1, v1, t2, v2

FP = mybir.dt.float32


def _block_starts(n, base=0):
    """128-row block start offsets covering [base, base+n), last block
    anchored at base+n-128 (overlap re-writes identical bytes)."""
    if n <= 0:
        return []
    if n <= P:
        return [base]
    starts = list(range(base, base + n - P, P))
    starts.append(base + n - P)
    return sorted(set(starts))


def _build_program(nu_max, n_rows, starts, m_starts, windows, trace_sim=False):
    nb = len(starts)
    wmax = max((wn for _, wn in windows), default=1)

    nc = bacc.Bacc(
        "TRN2",
        target_bir_lowering=False,
        debug=False,
        num_devices=N_CORES,
    )

    nu_buf = max(nu_max, P)  # host pads packed rows to >=128
    attn_d = nc.dram_tensor("attn", [nu_buf, H_PER, L], FP, kind="ExternalInput").ap()
    vecs_d = nc.dram_tensor("vecs", [P, nb * K], FP, kind="ExternalInput").ap()
    # rowconsts[0] = c_sink, [1] = maskval, [2] = iota
    rowconsts_d = nc.dram_tensor("rowconsts", [3, L], FP, kind="ExternalInput").ap()
    out_d = nc.dram_tensor("out", [n_rows, H_PER, L], FP, kind="ExternalOutput").ap()

    attn_rr = attn_d.rearrange("r h c -> r (h c)")
    out_rr = out_d.rearrange("r h c -> r (h c)")

    with tile.TileContext(nc, trace_sim=trace_sim) as tc, ExitStack() as ctx:
        const_pool = ctx.enter_context(tc.tile_pool(name="const", bufs=1))
        prep_pool = ctx.enter_context(tc.tile_pool(name="prep", bufs=2))
        band_pool = ctx.enter_context(tc.tile_pool(name="band", bufs=1))
        a_pool = ctx.enter_context(tc.tile_pool(name="a", bufs=3))

        cs_row = const_pool.tile([1, L], FP, tag="cs_row")
        nc.sync.dma_start(out=cs_row[:, :], in_=rowconsts_d[0:1, :])
        mv_row = const_pool.tile([1, L], FP, tag="mv_row")
        nc.sync.dma_start(out=mv_row[:, :], in_=rowconsts_d[1:2, :])
        io_row = const_pool.tile([1, L], FP, tag="io_row")
        nc.sync.dma_start(out=io_row[:, :], in_=rowconsts_d[2:3, :])
        vecs_sb = const_pool.tile([P, nb * K], FP, tag="vecs")
        nc.sync.dma_start(out=vecs_sb[:, :], in_=vecs_d[:, :])
        csink_bc = const_pool.tile([P, L], FP, tag="csink_bc")
        nc.gpsimd.partition_broadcast(csink_bc[:, :], cs_row[0:1, :])
        maskval_bc = const_pool.tile([P, L], FP, tag="maskval_bc")
        nc.gpsimd.partition_broadcast(maskval_bc[:, :], mv_row[0:1, :])
        iota_bc = const_pool.tile([P, L], FP, tag="iota_bc")
        nc.gpsimd.partition_broadcast(iota_bc[:, :], io_row[0:1, :])

        # constant NEG tile for masked-row output blocks; [128, 2L] halves
        # keep SBUF small while every store stays 128-partition wide
        if m_starts:
            neg_t = const_pool.tile([P, 2 * L], FP, tag="neg")
            nc.gpsimd.memset(neg_t[:, :], float(NEG))
            for r0 in m_starts:
                for half in range(2):
                    nc.gpsimd.dma_start(
                        out=out_rr[r0 : r0 + P, half * 2 * L : (half + 1) * 2 * L],
                        in_=neg_t[:, :],
                    )

        for r, k0 in enumerate(starts):
            cs_col = vecs_sb[:, K * r + 0 : K * r + 1]
            t1_col = vecs_sb[:, K * r + 1 : K * r + 2]
            v1_col = vecs_sb[:, K * r + 2 : K * r + 3]
            t2_col = vecs_sb[:, K * r + 3 : K * r + 4]
            v2_col = vecs_sb[:, K * r + 4 : K * r + 5]
            ws, wn = windows[r]

            # sink bias, bitwise-matching reference: round(si*sj) then *BETA
            bias_t = prep_pool.tile([P, L], FP, tag="bias")
            nc.scalar.activation(
                out=bias_t[:, :],
                in_=csink_bc[:, :],
                func=mybir.ActivationFunctionType.Copy,
                scale=cs_col,
            )
            nc.scalar.activation(
                out=bias_t[:, :],
                in_=bias_t[:, :],
                func=mybir.ActivationFunctionType.Copy,
                scale=float(BETA),
            )
            # neighbor band: row i contributes sub[i]@col i-1, sup[i]@col i+1.
            # Packed rows are sorted, so cols live in a narrow window.
            if wn > 0:
                band1 = band_pool.tile([P, wmax], FP, tag="band1")
                nc.vector.tensor_scalar(
                    out=band1[:, :wn],
                    in0=iota_bc[:, ws : ws + wn],
                    scalar1=t1_col,
                    scalar2=v1_col,
                    op0=mybir.AluOpType.is_equal,
                    op1=mybir.AluOpType.mult,
                )
                band2 = band_pool.tile([P, wmax], FP, tag="band2")
                nc.vector.tensor_scalar(
                    out=band2[:, :wn],
                    in0=iota_bc[:, ws : ws + wn],
                    scalar1=t2_col,
                    scalar2=v2_col,
                    op0=mybir.AluOpType.is_equal,
                    op1=mybir.AluOpType.mult,
                )
                bias_win = bias_t[:, ws : ws + wn]
                nc.vector.tensor_tensor(
                    out=bias_win, in0=bias_win, in1=band1[:, :wn],
                    op=mybir.AluOpType.add,
                )
                nc.vector.tensor_tensor(
                    out=bias_win, in0=bias_win, in1=band2[:, :wn],
                    op=mybir.AluOpType.add,
                )

            a_t = a_pool.tile([P, H_PER * L], FP, tag="a")
            nc.sync.dma_start(out=a_t[:, :], in_=attn_rr[k0 : k0 + P, :])
            for h in range(H_PER):
                a_h = a_t[:, h * L : (h + 1) * L]
                nc.vector.tensor_tensor(
                    out=a_h, in0=a_h, in1=bias_t[:, :], op=mybir.AluOpType.add
                )
                nc.vector.tensor_tensor(
                    out=a_h, in0=a_h, in1=maskval_bc[:, :], op=mybir.AluOpType.min
                )
            nc.scalar.dma_start(out=out_rr[k0 : k0 + P, :], in_=a_t[:, :])

    nc.compile()
    return nc


def _band_vecs(c_local_b):
    """Per-row band values, bitwise-matching the reference's overlapping
    slice assignments."""
    sub = np.zeros(L, np.float32)
    sub[1] = c_local_b[1]
    sub[L - 1] = c_local_b[L - 1]
    sub[2 : L - 1] = c_local_b[1 : L - 2]
    sup = np.zeros(L, np.float32)
    sup[: L - 1] = c_local_b[1:]
    return ALPHA * sub, ALPHA * sup


def _host_prep(attn_scores, c_local, c_sink, mask):
    attn_scores = np.asarray(attn_scores, dtype=np.float32)
    c_local = np.asarray(c_local, dtype=np.float32)
    c_sink = np.asarray(c_sink, dtype=np.float32)
    mask = np.asarray(mask, dtype=bool)

    unm = [np.flatnonzero(~mask[b]) for b in range(B)]
    msk = [np.flatnonzero(mask[b]) for b in range(B)]
    nu = [len(u) for u in unm]
    nu_max = max(nu)
    nm_max = max(L - n for n in nu)
    nu_buf = max(nu_max, P)
    starts = _block_starts(nu_max)
    neg_base = nu_max if nu_max > P else (P if nu_max else 0)
    m_starts = _block_starts(max(nm_max, P if nm_max else 0), base=neg_base)
    n_rows = (m_starts[-1] + P) if m_starts else (starts[-1] + P if starts else 0)
    nb = len(starts)

    # padded permutations (junk rows read real data; host ignores them)
    pis = []
    for b in range(B):
        pi = np.empty(nu_buf, np.int64)
        pi[: nu[b]] = unm[b]
        if nu[b] < nu_buf:
            pi[nu[b] :] = unm[b][-1] if nu[b] else 0
        pis.append(pi)

    # band windows per block: union over batches so one program fits both
    windows = []
    for k0 in starts:
        ws, we = L, -1
        for b in range(B):
            blk = pis[b][k0 : k0 + P]
            ws = min(ws, max(int(blk.min()) - 1, 0))
            we = max(we, min(int(blk.max()) + 1, L - 1))
        windows.append((ws, we - ws + 1))

    in_maps = []
    for c in range(N_CORES):
        b = c // (N_CORES // B)
        h0 = H_PER * (c % (N_CORES // B))
        pi = pis[b]
        sub, sup = _band_vecs(c_local[b])

        vecs = np.zeros((P, nb * K), np.float32)
        for r, k0 in enumerate(starts):
            rows = pi[k0 : k0 + P]
            vecs[:, K * r + 0] = c_sink[b][rows]
            vecs[:, K * r + 1] = (rows - 1).astype(np.float32)
            vecs[:, K * r + 2] = sub[rows]
            vecs[:, K * r + 3] = (rows + 1).astype(np.float32)
            vecs[:, K * r + 4] = sup[rows]

        maskval = np.where(mask[b], NEG, BIG).astype(np.float32)
        rowconsts = np.stack(
            [c_sink[b], maskval, np.arange(L, dtype=np.float32)], axis=0
        )
        # packed rows, head-transposed: [nu_buf, H_PER, L]
        attn_packed = np.ascontiguousarray(
            attn_scores[b, h0 : h0 + H_PER][:, pi, :].transpose(1, 0, 2)
        )
        in_maps.append(
            {
                "attn": attn_packed,
                "vecs": np.ascontiguousarray(vecs),
                "rowconsts": np.ascontiguousarray(rowconsts),
            }
        )
    shape_key = (nu_max, n_rows, tuple(starts), tuple(m_starts), tuple(windows))
    return in_maps, shape_key, (unm, msk, nu, neg_base)


_PROGRAM_CACHE = {}


def kernel(attn_scores, c_local, c_sink, mask, _trace=False, _trace_kwargs=None):
    in_maps, shape_key, (unm, msk, nu, neg_base) = _host_prep(
        attn_scores, c_local, c_sink, mask
    )
    nu_max, n_rows, starts, m_starts, windows = shape_key
    if shape_key not in _PROGRAM_CACHE:
        _PROGRAM_CACHE.clear()
        _PROGRAM_CACHE[shape_key] = _build_program(
            nu_max, n_rows, list(starts), list(m_starts), list(windows)
        )
    nc = _PROGRAM_CACHE[shape_key]

    res = run_bass_kernel_spmd(
        nc,
        in_maps,
        list(range(N_CORES)),
        trace=_trace,
        **(_trace_kwargs or {}),
    )
    out = np.empty((B, H, L, L), dtype=np.float32)
    for c in range(N_CORES):
        b = c // (N_CORES // B)
        h0 = H_PER * (c % (N_CORES // B))
        dev = res.results[c]["out"]  # [n_rows, H_PER, L]
        out[b, h0 : h0 + H_PER, unm[b], :] = dev[: nu[b]]
        out[b, h0 : h0 + H_PER, msk[b], :] = dev[neg_base : neg_base + (L - nu[b])]
    kernel.last_results = res
    return out
